# revision 20
# baseline (speedup 1.0000x reference)
"""PinSAGE-style sampled-neighbor mean + linear on 8 Trainium2 NeuronCores.

Strategy: shard the 100k output nodes across the 8 cores (12.5k each,
padded to 12800 = 10 chunks x 1280 nodes); x is replicated as an fp16 copy
with a leading all-zero row. The random 128-float-row gather dominates:
per-row DMA descriptors can only be generated by the Q7 SWDGE ucode
(~8ns/descriptor on one queue), so all gathers use the batched
InstDMAGatherAnt path spread over all 4 SWDGE queue contexts (measured
~3ns/index effective). dma_gather's int16 row index forces a 4-bank split
of x; hop1 gathers each chunk's unique source rows bank-compacted, bounces
them to an HBM scratch tile (cheap contiguous per-bank stores), and a
second 4-queue HBM dma_gather re-expands them into group-major
node-aligned order. (The SBUF-source transpose gather would avoid the
bounce but corrupts data when two run concurrently; the HBM path is
concurrency-safe.) hop1 of chunk t+1 is issued before hop2 of chunk t so
the queue contexts never starve while the engine waits on the bounce. Per
128-node group the tap-sum is a contiguous fp16 add tree on DVE, then PE
transpose, fp16 matmul with W.T, scale by 1/count + bias, and an fp16
HWDGE store (upcast to f32 on the host).

Host-side prep only builds index/scale tables (the "sampling" step):
first-10-edges-per-node selection, bank bucketing, token maps, and the
fp16 cast of x/W.
"""

import numpy as np

N_NODES = 100000
N_EDGES = 1600000
D = 128
TAPS = 10
N_CORES = 8
NODES_PER_CORE = 12500
NODES_PAD = 12800
N_CHUNKS = 5
CHUNK_NODES = 2560                 # nodes per chunk = 20 groups of 128
GPC = CHUNK_NODES // 128           # groups per chunk
CHUNK_SLOTS = CHUNK_NODES * TAPS   # 25600
BANK = 25088                       # balanced bank size; 4*25088 = 100352
N_BANKS = 4
X16_ROWS = 100352                  # 1 zero row + 100000 rows + pad; 4 banks
HOP2_SPLIT = 2                     # hop2 split into 2 calls of 12800 idx

_cache = {}


def _build_tables(edge_index):
    """First-TAPS-edges-per-node neighbor lists with torch-masking semantics.

    Returns idx [N,TAPS] (-1 padded), cnt_eff [N], inv [N] f32.
    Nodes with no out-edges get a single self tap.
    """
    row = np.asarray(edge_index[0], dtype=np.int64)
    col = np.asarray(edge_index[1], dtype=np.int64)
    E = row.shape[0]
    order = np.argsort(row, kind="stable")
    row_s = row[order]
    col_s = col[order]
    starts = np.searchsorted(row_s, np.arange(N_NODES, dtype=np.int64))
    counts = np.diff(np.append(starts, E))
    rank = np.arange(E, dtype=np.int64) - starts[row_s]
    keep = rank < TAPS
    kr = row_s[keep]
    kc = col_s[keep]
    krank = rank[keep]

    idx = np.full((N_NODES, TAPS), -1, np.int64)
    idx[kr, krank] = kc
    cnt = np.minimum(counts, TAPS)
    self_nodes = cnt == 0
    idx[self_nodes, 0] = np.nonzero(self_nodes)[0]
    cnt_eff = np.maximum(cnt, 1)
    inv = (1.0 / cnt_eff).astype(np.float32)
    return idx.astype(np.int32), cnt_eff.astype(np.int32), inv


def _plan(edge_index):
    """Per-core per-chunk bank-compact gather lists and hop2 token maps.

    hop1 gathers the chunk's UNIQUE source rows (hop2 re-expands duplicates
    for free); per-(chunk, bank) static bounds are the max over cores.
    """
    idx_all, cnt_all, inv_all = _build_tables(edge_index)

    counts = np.zeros((N_CORES, N_CHUNKS, N_BANKS), np.int64)
    core_data = []
    for c in range(N_CORES):
        lo = c * NODES_PER_CORE
        chunks = []
        for t in range(N_CHUNKS):
            ln0 = t * CHUNK_NODES
            n_real = max(0, min(CHUNK_NODES, NODES_PER_CORE - ln0))
            g = np.arange(lo + ln0, lo + ln0 + n_real)
            nbr = idx_all[g]                       # [n_real, TAPS] (-1 pad)
            nn, jj = np.nonzero(nbr >= 0)
            src = nbr[nn, jj] + 1                  # +1: row 0 is the zero row
            # group-major slots: slot = gl*1280 + j*128 + p
            # -> dst partition p, column-unit gl*10+j
            slot = (nn // 128) * 1280 + jj * 128 + (nn % 128)
            uniq, invm = np.unique(src, return_inverse=True)
            ubank = uniq // BANK
            chunks.append((slot, uniq, invm, ubank, n_real))
            for b in range(N_BANKS):
                counts[c, t, b] = int((ubank == b).sum())
        core_data.append(chunks)

    # per-(chunk, bank) bounds: max over cores, zero-token slot in bank 0,
    # rounded to multiples of 128 (column units)
    bounds = np.zeros((N_CHUNKS, N_BANKS), np.int64)
    for t in range(N_CHUNKS):
        for b in range(N_BANKS):
            m = int(counts[:, t, b].max()) + (1 if b == 0 else 0)
            bounds[t, b] = int(np.ceil(max(m, 4) / 128.0)) * 128
    regions = np.zeros((N_CHUNKS, N_BANKS + 1), np.int64)
    regions[:, 1:] = np.cumsum(bounds, axis=1)
    widths = regions[:, -1].copy()                 # Gc slots / GcH rows
    assert int(widths.max()) <= 32767, widths.max()
    wmax = int(widths.max())

    hop1 = np.zeros((N_CORES, N_CHUNKS, 128, wmax // 16), np.int16)
    hop2 = np.zeros((N_CORES, N_CHUNKS, 128, CHUNK_SLOTS // 16), np.int16)
    invA = np.ones((N_CORES, N_CHUNKS, 128, GPC), np.float32)

    ar = np.arange
    ii2 = ar(CHUNK_SLOTS)
    for c in range(N_CORES):
        for t in range(N_CHUNKS):
            slot, uniq, invm, ubank, n_real = core_data[c][t]
            width = int(widths[t])
            w_cu = width // 128
            h1 = np.zeros(width, np.int16)
            tok_of_k = np.zeros(len(uniq), np.int64)
            for b in range(N_BANKS):
                base = int(regions[t, b])
                sel = ubank == b
                s_src = (uniq[sel] - b * BANK).astype(np.int64)
                off = 1 if b == 0 else 0           # pos 0 of bank0 = zero row
                pos = base + off + ar(len(s_src))
                h1[pos] = s_src.astype(np.int16)
                # GcH row of pos: (pos%128)*w_cu + pos//128
                tok_of_k[sel] = (pos % 128) * w_cu + pos // 128
            tok = np.zeros(CHUNK_SLOTS, np.int64)  # default: zero token
            tok[slot] = tok_of_k[invm]
            ii = ar(width)
            for gg in range(8):
                hop1[c, t, 16 * gg + ii % 16, ii // 16] = h1
                hop2[c, t, 16 * gg + ii2 % 16, ii2 // 16] = \
                    tok.astype(np.int16)
            lo = c * NODES_PER_CORE + t * CHUNK_NODES
            iv = np.ones(CHUNK_NODES, np.float32)
            iv[:n_real] = inv_all[lo:lo + n_real]
            invA[c, t] = iv.reshape(GPC, 128).T
    return {"bounds": bounds, "regions": regions, "widths": widths,
            "wmax": wmax, "hop1": hop1, "hop2": hop2, "inv": invA}


def _build_program(bounds, regions, widths, wmax):
    import concourse.bass as bass
    import concourse.mybir as mybir
    import concourse.tile as tile
    from concourse import bacc
    from concourse.library_config import mlp
    from concourse.masks import make_identity

    nc = bacc.Bacc("TRN2", target_bir_lowering=False, debug=False,
                   enable_asserts=True, num_devices=N_CORES,
                   num_swdge_queues=4)
    x16 = nc.dram_tensor("x16", [X16_ROWS, D], mybir.dt.float16,
                         kind="ExternalInput").ap()
    idx1 = nc.dram_tensor("idx1", [N_CHUNKS, 128, wmax // 16],
                          mybir.dt.int16, kind="ExternalInput").ap()
    idx2 = nc.dram_tensor("idx2", [N_CHUNKS, 128, CHUNK_SLOTS // 16],
                          mybir.dt.int16, kind="ExternalInput").ap()
    inv = nc.dram_tensor("inv", [N_CHUNKS, 128, GPC],
                         mybir.dt.float32, kind="ExternalInput").ap()
    wt16 = nc.dram_tensor("wt16", [D, D], mybir.dt.float16,
                          kind="ExternalInput").ap()
    bias_rep = nc.dram_tensor("bias_rep", [128, D], mybir.dt.float32,
                              kind="ExternalInput").ap()
    out = nc.dram_tensor("out", [NODES_PAD, D], mybir.dt.float16,
                         kind="ExternalOutput").ap()

    qctr = [0]

    def nextq():
        q = qctr[0] % 4
        qctr[0] += 1
        return q

    with tile.TileContext(nc) as tc:
        with tc.tile_pool(name="const", bufs=1) as const_p, \
             tc.tile_pool(name="meta", bufs=3) as meta_p, \
             tc.tile_pool(name="gc", bufs=2) as gc_p, \
             tc.tile_pool(name="gchbm", bufs=2, space="DRAM") as gch_p, \
             tc.tile_pool(name="gt", bufs=1) as gt_p, \
             tc.tile_pool(name="red", bufs=4) as red_p, \
             tc.tile_pool(name="outp", bufs=4) as out_p, \
             tc.tile_pool(name="ps1", bufs=2, space="PSUM") as ps1_p, \
             tc.tile_pool(name="ps2", bufs=2, space="PSUM") as ps2_p:

            nc.gpsimd.load_library(mlp)
            state = {}

            def emit_meta(t):
                width = int(widths[t])
                i1 = meta_p.tile([128, width // 16], mybir.dt.int16,
                                 name="i1")
                nc.sync.dma_start(i1[:], idx1[t][:, :width // 16])
                i2 = meta_p.tile([128, CHUNK_SLOTS // 16], mybir.dt.int16,
                                 name="i2")
                nc.sync.dma_start(i2[:], idx2[t])
                iv = meta_p.tile([128, GPC], mybir.dt.float32,
                                 name="iv")
                nc.sync.dma_start(iv[:], inv[t])
                return i1, i2, iv

            def emit_hop1(t, i1):
                # hop1: 4 bank-compact gathers from HBM into Gc, each bank
                # region bounced to HBM scratch right after its gather
                # (row r = p*w_cu + cu)
                width = int(widths[t])
                Gc = gc_p.tile([128, width], mybir.dt.float16, name="Gc")
                GcH = gch_p.tile([128, width], mybir.dt.float16, name="GcH")
                for b in range(N_BANKS):
                    nb = int(bounds[t, b])
                    base = int(regions[t, b])
                    o = Gc[:, base:base + nb].rearrange(
                        "p (c d) -> p c d", d=128)
                    rows = min(BANK, X16_ROWS - b * BANK)
                    nc.gpsimd.dma_gather(
                        o, x16[b * BANK:b * BANK + rows],
                        i1[:, base // 16:(base + nb) // 16],
                        nb, nb, 128,
                        single_packet=False, queue_num=nextq())
                    nc.sync.dma_start(GcH[:, base:base + nb],
                                      Gc[:, base:base + nb])
                return Gc, GcH

            def emit_rest(t, GcH, i2, iv):
                width = int(widths[t])
                ident = state["ident"]
                wt_sb = state["wt_sb"]
                bias_sb = state["bias_sb"]
                # hop2: HBM-path re-gather to group-major node-aligned
                # order; 5 calls of 2 groups each into separate tiles so
                # group compute starts as soon as its slice lands
                gch_rows = GcH[:].rearrange("p (c d) -> (p c) d", d=128)
                ns = CHUNK_SLOTS // HOP2_SPLIT
                GTs = []
                for k in range(HOP2_SPLIT):
                    GTk = gt_p.tile([128, ns], mybir.dt.float16,
                                    name=f"GT{k}")
                    GTs.append(GTk)
                    o2 = GTk[:].rearrange("p (c d) -> p c d", d=128)
                    nc.gpsimd.dma_gather(
                        o2, gch_rows,
                        i2[:, k * ns // 16:(k + 1) * ns // 16],
                        ns, ns, 128,
                        single_packet=False, queue_num=nextq())
                # GT column-unit cu = gl*10 + j holds tap j of group gl,
                # node gl*128+p on partition p
                for gl in range(GPC):
                    g = t * GPC + gl
                    GTk = GTs[gl * 1280 // ns]
                    lo_c = gl * 1280 - (gl * 1280 // ns) * ns
                    A = GTk[:, lo_c:lo_c + 1280]
                    # tap-sum via contiguous fp16 adds: 5+5 -> (2+2)+1
                    T1 = red_p.tile([128, 640], mybir.dt.float16, name="T1")
                    nc.vector.tensor_tensor(
                        out=T1[:], in0=A[:, 0:640], in1=A[:, 640:1280],
                        op=mybir.AluOpType.add)
                    T2 = red_p.tile([128, 256], mybir.dt.float16, name="T2")
                    nc.vector.tensor_tensor(
                        out=T2[:], in0=T1[:, 0:256], in1=T1[:, 256:512],
                        op=mybir.AluOpType.add)
                    T3 = red_p.tile([128, 128], mybir.dt.float16, name="T3")
                    nc.vector.tensor_tensor(
                        out=T3[:], in0=T2[:, 0:128], in1=T2[:, 128:256],
                        op=mybir.AluOpType.add)
                    S = red_p.tile([128, 128], mybir.dt.float32, name="S")
                    nc.vector.tensor_tensor(
                        out=S[:], in0=T3[:], in1=T1[:, 512:640],
                        op=mybir.AluOpType.add)
                    # psum1[d, n] = S^T (PE transpose)
                    psum1 = ps1_p.tile([128, 128], mybir.dt.float32,
                                       space="PSUM", name="psum1")
                    nc.tensor.matmul(psum1[:], lhsT=S[:], rhs=ident[:],
                                     is_transpose=True, start=True,
                                     stop=True)
                    sT = red_p.tile([128, 128], mybir.dt.float16, name="sT")
                    nc.scalar.copy(sT[:], psum1[:])
                    psum2 = ps2_p.tile([128, 128], mybir.dt.float32,
                                       space="PSUM", name="psum2")
                    nc.tensor.matmul(psum2[:], lhsT=sT[:], rhs=wt_sb[:],
                                     start=True, stop=True)
                    o_sb = out_p.tile([128, D], mybir.dt.float16,
                                      name="o_sb")
                    nc.vector.scalar_tensor_tensor(
                        out=o_sb[:],
                        in0=psum2[:],
                        scalar=iv[:, gl:gl + 1],
                        in1=bias_sb[:],
                        op0=mybir.AluOpType.mult,
                        op1=mybir.AluOpType.add,
                    )
                    nc.sync.dma_start(out[g * 128:(g + 1) * 128, :],
                                      o_sb[:])

            # chunk 0's gathers go first; consts load in their shadow.
            # hop1(t+1) is issued before hop2(t) so the queues have
            # independent work while the engine waits on the bounce sem.
            metas = {0: emit_meta(0)}
            Gc_cur, GcH_cur = emit_hop1(0, metas[0][0])
            ident = const_p.tile([128, 128], mybir.dt.float32)
            make_identity(nc, ident[:])
            wt_sb = const_p.tile([D, D], mybir.dt.float16)
            nc.sync.dma_start(wt_sb[:], wt16[:])
            bias_sb = const_p.tile([128, D], mybir.dt.float32)
            nc.sync.dma_start(bias_sb[:], bias_rep[:])
            state.update(ident=ident, wt_sb=wt_sb, bias_sb=bias_sb)
            for t in range(N_CHUNKS):
                GcH = GcH_cur
                if t + 1 < N_CHUNKS:
                    metas[t + 1] = emit_meta(t + 1)
                    Gc_cur, GcH_cur = emit_hop1(t + 1, metas[t + 1][0])
                emit_rest(t, GcH, metas[t][1], metas[t][2])
                del metas[t]
    nc.compile()
    return nc


def _build_in_maps(x, edge_index, W, b):
    x = np.asarray(x, dtype=np.float32)
    W = np.asarray(W, dtype=np.float32)
    b = np.asarray(b, dtype=np.float32)

    plan = _plan(edge_index)

    x16 = np.zeros((X16_ROWS, D), np.float16)
    x16[1:N_NODES + 1] = x.astype(np.float16)
    wt16_host = np.ascontiguousarray(W.T.astype(np.float16))
    bias_host = np.ascontiguousarray(
        np.broadcast_to(b[None, :], (128, D)).astype(np.float32))

    in_maps = []
    for c in range(N_CORES):
        in_maps.append({
            "x16": x16,
            "idx1": np.ascontiguousarray(plan["hop1"][c]),
            "idx2": np.ascontiguousarray(plan["hop2"][c]),
            "inv": np.ascontiguousarray(plan["inv"][c]),
            "wt16": wt16_host,
            "bias_rep": bias_host,
        })
    return in_maps, plan


def kernel(x, edge_index, W, b):
    from concourse.bass_utils import run_bass_kernel_spmd

    in_maps, plan = _build_in_maps(x, edge_index, W, b)

    key = tuple(plan["bounds"].ravel().tolist())
    if _cache.get("key") != key:
        _cache["nc"] = _build_program(plan["bounds"], plan["regions"],
                                      plan["widths"], plan["wmax"])
        _cache["key"] = key
    nc = _cache["nc"]

    res = run_bass_kernel_spmd(nc, in_maps, core_ids=list(range(N_CORES)))
    outs = [res.results[c]["out"][:NODES_PER_CORE].astype(np.float32)
            for c in range(N_CORES)]
    return np.concatenate(outs, axis=0)


# revision 22
# speedup vs baseline: 1.5421x; 1.5421x over previous
"""PinSAGE-style sampled-neighbor mean + linear on 8 Trainium2 NeuronCores.

Strategy: shard the 100k output nodes across the 8 cores (12.5k each,
padded to 12800 = 10 chunks x 1280 nodes); x is replicated as an fp16 copy
with a leading all-zero row. The random 128-float-row gather dominates:
per-row DMA descriptors can only be generated by the Q7 SWDGE ucode
(~8ns/descriptor on one queue), so all gathers use the batched
InstDMAGatherAnt path spread over all 4 SWDGE queue contexts (measured
~3ns/index effective). dma_gather's int16 row index forces a 4-bank split
of x; hop1 gathers each chunk's unique source rows bank-compacted, bounces
them to an HBM scratch tile (cheap contiguous per-bank stores), and a
second 4-queue HBM dma_gather re-expands them into group-major
node-aligned order. (The SBUF-source transpose gather would avoid the
bounce but corrupts data when two run concurrently; the HBM path is
concurrency-safe.) hop1 of chunk t+1 is issued before hop2 of chunk t so
the queue contexts never starve while the engine waits on the bounce. Per
128-node group the tap-sum is a contiguous fp16 add tree on DVE, then PE
transpose, fp16 matmul with W.T, scale by 1/count + bias, and an fp16
HWDGE store (upcast to f32 on the host).

Host-side prep only builds index/scale tables (the "sampling" step):
first-10-edges-per-node selection, bank bucketing, token maps, and the
fp16 cast of x/W.
"""

import numpy as np

N_NODES = 100000
N_EDGES = 1600000
D = 128
TAPS = 10
N_CORES = 8
NODES_PER_CORE = 12500
NODES_PAD = 12800
N_CHUNKS = 5
CHUNK_NODES = 2560                 # nodes per chunk = 20 groups of 128
GPC = CHUNK_NODES // 128           # groups per chunk
CHUNK_SLOTS = CHUNK_NODES * TAPS   # 25600
BANK = 25088                       # balanced bank size; 4*25088 = 100352
N_BANKS = 4
X16_ROWS = 100352                  # 1 zero row + 100000 rows + pad; 4 banks
HOP2_SPLIT = 5                     # hop2 split into 5 calls of 5120 idx

_cache = {}


def _build_tables(edge_index):
    """First-TAPS-edges-per-node neighbor lists with torch-masking semantics.

    Returns idx [N,TAPS] (-1 padded), cnt_eff [N], inv [N] f32.
    Nodes with no out-edges get a single self tap.
    """
    row = np.asarray(edge_index[0], dtype=np.int64)
    col = np.asarray(edge_index[1], dtype=np.int64)
    E = row.shape[0]
    order = np.argsort(row, kind="stable")
    row_s = row[order]
    col_s = col[order]
    starts = np.searchsorted(row_s, np.arange(N_NODES, dtype=np.int64))
    counts = np.diff(np.append(starts, E))
    rank = np.arange(E, dtype=np.int64) - starts[row_s]
    keep = rank < TAPS
    kr = row_s[keep]
    kc = col_s[keep]
    krank = rank[keep]

    idx = np.full((N_NODES, TAPS), -1, np.int64)
    idx[kr, krank] = kc
    cnt = np.minimum(counts, TAPS)
    self_nodes = cnt == 0
    idx[self_nodes, 0] = np.nonzero(self_nodes)[0]
    cnt_eff = np.maximum(cnt, 1)
    inv = (1.0 / cnt_eff).astype(np.float32)
    return idx.astype(np.int32), cnt_eff.astype(np.int32), inv


def _plan(edge_index):
    """Per-core per-chunk bank-compact gather lists and hop2 token maps.

    hop1 gathers the chunk's UNIQUE source rows (hop2 re-expands duplicates
    for free); per-(chunk, bank) static bounds are the max over cores.
    """
    idx_all, cnt_all, inv_all = _build_tables(edge_index)

    counts = np.zeros((N_CORES, N_CHUNKS, N_BANKS), np.int64)
    core_data = []
    for c in range(N_CORES):
        lo = c * NODES_PER_CORE
        chunks = []
        for t in range(N_CHUNKS):
            ln0 = t * CHUNK_NODES
            n_real = max(0, min(CHUNK_NODES, NODES_PER_CORE - ln0))
            g = np.arange(lo + ln0, lo + ln0 + n_real)
            nbr = idx_all[g]                       # [n_real, TAPS] (-1 pad)
            nn, jj = np.nonzero(nbr >= 0)
            src = nbr[nn, jj] + 1                  # +1: row 0 is the zero row
            # group-major slots: slot = gl*1280 + j*128 + p
            # -> dst partition p, column-unit gl*10+j
            slot = (nn // 128) * 1280 + jj * 128 + (nn % 128)
            uniq, invm = np.unique(src, return_inverse=True)
            ubank = uniq // BANK
            chunks.append((slot, uniq, invm, ubank, n_real))
            for b in range(N_BANKS):
                counts[c, t, b] = int((ubank == b).sum())
        core_data.append(chunks)

    # per-(chunk, bank) bounds: max over cores, zero-token slot in bank 0,
    # rounded to multiples of 128 (column units)
    bounds = np.zeros((N_CHUNKS, N_BANKS), np.int64)
    for t in range(N_CHUNKS):
        for b in range(N_BANKS):
            m = int(counts[:, t, b].max()) + (1 if b == 0 else 0)
            bounds[t, b] = int(np.ceil(max(m, 4) / 128.0)) * 128
    regions = np.zeros((N_CHUNKS, N_BANKS + 1), np.int64)
    regions[:, 1:] = np.cumsum(bounds, axis=1)
    widths = regions[:, -1].copy()                 # Gc slots / GcH rows
    assert int(widths.max()) <= 32767, widths.max()
    wmax = int(widths.max())

    hop1 = np.zeros((N_CORES, N_CHUNKS, 128, wmax // 16), np.int16)
    hop2 = np.zeros((N_CORES, N_CHUNKS, 128, CHUNK_SLOTS // 16), np.int16)
    invA = np.ones((N_CORES, N_CHUNKS, 128, GPC), np.float32)

    ar = np.arange
    ii2 = ar(CHUNK_SLOTS)
    for c in range(N_CORES):
        for t in range(N_CHUNKS):
            slot, uniq, invm, ubank, n_real = core_data[c][t]
            width = int(widths[t])
            w_cu = width // 128
            h1 = np.zeros(width, np.int16)
            tok_of_k = np.zeros(len(uniq), np.int64)
            for b in range(N_BANKS):
                base = int(regions[t, b])
                sel = ubank == b
                s_src = (uniq[sel] - b * BANK).astype(np.int64)
                off = 1 if b == 0 else 0           # pos 0 of bank0 = zero row
                pos = base + off + ar(len(s_src))
                h1[pos] = s_src.astype(np.int16)
                # GcH row of pos: (pos%128)*w_cu + pos//128
                tok_of_k[sel] = (pos % 128) * w_cu + pos // 128
            tok = np.zeros(CHUNK_SLOTS, np.int64)  # default: zero token
            tok[slot] = tok_of_k[invm]
            ii = ar(width)
            for gg in range(8):
                hop1[c, t, 16 * gg + ii % 16, ii // 16] = h1
                hop2[c, t, 16 * gg + ii2 % 16, ii2 // 16] = \
                    tok.astype(np.int16)
            lo = c * NODES_PER_CORE + t * CHUNK_NODES
            iv = np.ones(CHUNK_NODES, np.float32)
            iv[:n_real] = inv_all[lo:lo + n_real]
            invA[c, t] = iv.reshape(GPC, 128).T
    return {"bounds": bounds, "regions": regions, "widths": widths,
            "wmax": wmax, "hop1": hop1, "hop2": hop2, "inv": invA}


def _build_program(bounds, regions, widths, wmax):
    import concourse.bass as bass
    import concourse.mybir as mybir
    import concourse.tile as tile
    from concourse import bacc
    from concourse.library_config import mlp
    from concourse.masks import make_identity

    nc = bacc.Bacc("TRN2", target_bir_lowering=False, debug=False,
                   enable_asserts=True, num_devices=N_CORES,
                   num_swdge_queues=4)
    x16 = nc.dram_tensor("x16", [X16_ROWS, D], mybir.dt.float16,
                         kind="ExternalInput").ap()
    idx1 = nc.dram_tensor("idx1", [N_CHUNKS, 128, wmax // 16],
                          mybir.dt.int16, kind="ExternalInput").ap()
    idx2 = nc.dram_tensor("idx2", [N_CHUNKS, 128, CHUNK_SLOTS // 16],
                          mybir.dt.int16, kind="ExternalInput").ap()
    inv = nc.dram_tensor("inv", [N_CHUNKS, 128, GPC],
                         mybir.dt.float32, kind="ExternalInput").ap()
    wt16 = nc.dram_tensor("wt16", [D, D], mybir.dt.float16,
                          kind="ExternalInput").ap()
    bias_rep = nc.dram_tensor("bias_rep", [128, D], mybir.dt.float32,
                              kind="ExternalInput").ap()
    out = nc.dram_tensor("out", [NODES_PAD, D], mybir.dt.float16,
                         kind="ExternalOutput").ap()

    qctr = [0]

    def nextq():
        q = qctr[0] % 4
        qctr[0] += 1
        return q

    with tile.TileContext(nc) as tc:
        with tc.tile_pool(name="const", bufs=1) as const_p, \
             tc.tile_pool(name="meta", bufs=3) as meta_p, \
             tc.tile_pool(name="gc", bufs=2) as gc_p, \
             tc.tile_pool(name="gchbm", bufs=2, space="DRAM") as gch_p, \
             tc.tile_pool(name="gt", bufs=1) as gt_p, \
             tc.tile_pool(name="red", bufs=4) as red_p, \
             tc.tile_pool(name="outp", bufs=4) as out_p, \
             tc.tile_pool(name="ps1", bufs=2, space="PSUM") as ps1_p, \
             tc.tile_pool(name="ps2", bufs=2, space="PSUM") as ps2_p:

            nc.gpsimd.load_library(mlp)
            state = {}

            def emit_meta(t):
                width = int(widths[t])
                i1 = meta_p.tile([128, width // 16], mybir.dt.int16,
                                 name="i1")
                nc.sync.dma_start(i1[:], idx1[t][:, :width // 16])
                i2 = meta_p.tile([128, CHUNK_SLOTS // 16], mybir.dt.int16,
                                 name="i2")
                nc.sync.dma_start(i2[:], idx2[t])
                iv = meta_p.tile([128, GPC], mybir.dt.float32,
                                 name="iv")
                nc.sync.dma_start(iv[:], inv[t])
                return i1, i2, iv

            def emit_hop1(t, i1):
                # hop1: 4 bank-compact gathers from HBM into Gc, each bank
                # region bounced to HBM scratch right after its gather
                # (row r = p*w_cu + cu)
                width = int(widths[t])
                Gc = gc_p.tile([128, width], mybir.dt.float16, name="Gc")
                GcH = gch_p.tile([128, width], mybir.dt.float16, name="GcH")
                for b in range(N_BANKS):
                    nb = int(bounds[t, b])
                    base = int(regions[t, b])
                    o = Gc[:, base:base + nb].rearrange(
                        "p (c d) -> p c d", d=128)
                    rows = min(BANK, X16_ROWS - b * BANK)
                    nc.gpsimd.dma_gather(
                        o, x16[b * BANK:b * BANK + rows],
                        i1[:, base // 16:(base + nb) // 16],
                        nb, nb, 128,
                        single_packet=False, queue_num=nextq())
                    nc.sync.dma_start(GcH[:, base:base + nb],
                                      Gc[:, base:base + nb])
                return Gc, GcH

            def emit_rest(t, GcH, i2, iv):
                width = int(widths[t])
                ident = state["ident"]
                wt_sb = state["wt_sb"]
                bias_sb = state["bias_sb"]
                # hop2: HBM-path re-gather to group-major node-aligned
                # order; 5 calls of 2 groups each into separate tiles so
                # group compute starts as soon as its slice lands
                gch_rows = GcH[:].rearrange("p (c d) -> (p c) d", d=128)
                ns = CHUNK_SLOTS // HOP2_SPLIT
                GTs = []
                for k in range(HOP2_SPLIT):
                    GTk = gt_p.tile([128, ns], mybir.dt.float16,
                                    name=f"GT{k}")
                    GTs.append(GTk)
                    o2 = GTk[:].rearrange("p (c d) -> p c d", d=128)
                    nc.gpsimd.dma_gather(
                        o2, gch_rows,
                        i2[:, k * ns // 16:(k + 1) * ns // 16],
                        ns, ns, 128,
                        single_packet=False, queue_num=nextq())
                # GT column-unit cu = gl*10 + j holds tap j of group gl,
                # node gl*128+p on partition p
                for gl in range(GPC):
                    g = t * GPC + gl
                    GTk = GTs[gl * 1280 // ns]
                    lo_c = gl * 1280 - (gl * 1280 // ns) * ns
                    A = GTk[:, lo_c:lo_c + 1280]
                    # tap-sum via contiguous fp16 adds: 5+5 -> (2+2)+1
                    T1 = red_p.tile([128, 640], mybir.dt.float16, name="T1")
                    nc.vector.tensor_tensor(
                        out=T1[:], in0=A[:, 0:640], in1=A[:, 640:1280],
                        op=mybir.AluOpType.add)
                    T2 = red_p.tile([128, 256], mybir.dt.float16, name="T2")
                    nc.vector.tensor_tensor(
                        out=T2[:], in0=T1[:, 0:256], in1=T1[:, 256:512],
                        op=mybir.AluOpType.add)
                    T3 = red_p.tile([128, 128], mybir.dt.float16, name="T3")
                    nc.vector.tensor_tensor(
                        out=T3[:], in0=T2[:, 0:128], in1=T2[:, 128:256],
                        op=mybir.AluOpType.add)
                    S = red_p.tile([128, 128], mybir.dt.float32, name="S")
                    nc.vector.tensor_tensor(
                        out=S[:], in0=T3[:], in1=T1[:, 512:640],
                        op=mybir.AluOpType.add)
                    # psum1[d, n] = S^T (PE transpose)
                    psum1 = ps1_p.tile([128, 128], mybir.dt.float32,
                                       space="PSUM", name="psum1")
                    nc.tensor.matmul(psum1[:], lhsT=S[:], rhs=ident[:],
                                     is_transpose=True, start=True,
                                     stop=True)
                    sT = red_p.tile([128, 128], mybir.dt.float16, name="sT")
                    nc.scalar.copy(sT[:], psum1[:])
                    psum2 = ps2_p.tile([128, 128], mybir.dt.float32,
                                       space="PSUM", name="psum2")
                    nc.tensor.matmul(psum2[:], lhsT=sT[:], rhs=wt_sb[:],
                                     start=True, stop=True)
                    o_sb = out_p.tile([128, D], mybir.dt.float16,
                                      name="o_sb")
                    nc.vector.scalar_tensor_tensor(
                        out=o_sb[:],
                        in0=psum2[:],
                        scalar=iv[:, gl:gl + 1],
                        in1=bias_sb[:],
                        op0=mybir.AluOpType.mult,
                        op1=mybir.AluOpType.add,
                    )
                    nc.sync.dma_start(out[g * 128:(g + 1) * 128, :],
                                      o_sb[:])

            # chunk 0's gathers go first; consts load in their shadow.
            # hop1(t+1) is issued before hop2(t) so the queues have
            # independent work while the engine waits on the bounce sem.
            metas = {0: emit_meta(0)}
            Gc_cur, GcH_cur = emit_hop1(0, metas[0][0])
            ident = const_p.tile([128, 128], mybir.dt.float32)
            make_identity(nc, ident[:])
            wt_sb = const_p.tile([D, D], mybir.dt.float16)
            nc.sync.dma_start(wt_sb[:], wt16[:])
            bias_sb = const_p.tile([128, D], mybir.dt.float32)
            nc.sync.dma_start(bias_sb[:], bias_rep[:])
            state.update(ident=ident, wt_sb=wt_sb, bias_sb=bias_sb)
            for t in range(N_CHUNKS):
                GcH = GcH_cur
                if t + 1 < N_CHUNKS:
                    metas[t + 1] = emit_meta(t + 1)
                    Gc_cur, GcH_cur = emit_hop1(t + 1, metas[t + 1][0])
                emit_rest(t, GcH, metas[t][1], metas[t][2])
                del metas[t]
    nc.compile()
    return nc


def _build_in_maps(x, edge_index, W, b):
    x = np.asarray(x, dtype=np.float32)
    W = np.asarray(W, dtype=np.float32)
    b = np.asarray(b, dtype=np.float32)

    plan = _plan(edge_index)

    x16 = np.zeros((X16_ROWS, D), np.float16)
    x16[1:N_NODES + 1] = x.astype(np.float16)
    wt16_host = np.ascontiguousarray(W.T.astype(np.float16))
    bias_host = np.ascontiguousarray(
        np.broadcast_to(b[None, :], (128, D)).astype(np.float32))

    in_maps = []
    for c in range(N_CORES):
        in_maps.append({
            "x16": x16,
            "idx1": np.ascontiguousarray(plan["hop1"][c]),
            "idx2": np.ascontiguousarray(plan["hop2"][c]),
            "inv": np.ascontiguousarray(plan["inv"][c]),
            "wt16": wt16_host,
            "bias_rep": bias_host,
        })
    return in_maps, plan


def kernel(x, edge_index, W, b):
    from concourse.bass_utils import run_bass_kernel_spmd

    in_maps, plan = _build_in_maps(x, edge_index, W, b)

    key = tuple(plan["bounds"].ravel().tolist())
    if _cache.get("key") != key:
        _cache["nc"] = _build_program(plan["bounds"], plan["regions"],
                                      plan["widths"], plan["wmax"])
        _cache["key"] = key
    nc = _cache["nc"]

    res = run_bass_kernel_spmd(nc, in_maps, core_ids=list(range(N_CORES)))
    outs = [res.results[c]["out"][:NODES_PER_CORE].astype(np.float32)
            for c in range(N_CORES)]
    return np.concatenate(outs, axis=0)
